# revision 16
# baseline (speedup 1.0000x reference)
"""AttentionBlock (ChannelNorm + MHA + proj + residual) Trainium2 Bass kernel.

Sharding: 8 cores = 4 batches x 2 head-groups. Core c handles batch c//2 and
heads [4*(c%2), 4*(c%2)+4). Each core computes LayerNorm + its slice of the
QKV projection + attention for its 4 heads + a partial proj_out contraction.
The host sums the two partials per batch and adds proj bias + residual.

All matmuls run in float32r (fast fp32 mode, ~1.5e-4 rel err). The whole
device pipeline works in a channels-on-partitions [C, L] layout so no
transposes are needed anywhere:
  - LN stats (sum, sum-sq over C) via ones-vector matmuls, rstd on DVE/ACT,
    per-position stats broadcast across partitions via 0-stride DMA.
  - q^T,k^T come out of the QKV GEMM as [d, L]; v as [L, d] — exactly the
    layouts the attention matmuls need.
  - scores are computed transposed (s^T[lk, lq]), softmax denominator comes
    free from a ones-column appended to v (M=65 matmul), normalization is
    folded in after the o^T accumulation.
"""
import numpy as np

import concourse.bass as bass
import concourse.mybir as mybir
import concourse.tile as tile
from concourse import bacc
from concourse.bass_utils import run_bass_kernel_spmd

F32 = mybir.dt.float32
F32R = mybir.dt.float32r

B, C, L, H = 4, 512, 2048, 8
DH = C // H          # 64
G = 2                # head groups (cores per batch)
HPC = H // G         # 4 heads per core
P = 128
KC = C // P          # 4 contraction chunks
NSTRIP = 4
STRIP = L // NSTRIP  # 512
LCH = L // P         # 16 l-chunks
SCALE = DH ** -0.5
EPS = 1e-5
ALU = mybir.AluOpType
ACTF = mybir.ActivationFunctionType


def build_nc():
    nc = bacc.Bacc()
    x_d = nc.dram_tensor("x_loc", [C, L], F32R, kind="ExternalInput")
    wqk_d = nc.dram_tensor("wqkT", [C, 2 * HPC * DH], F32R, kind="ExternalInput")
    wv_d = nc.dram_tensor("wvT", [C, HPC * DH], F32R, kind="ExternalInput")
    wp_d = nc.dram_tensor("wprojT", [2 * P, C], F32R, kind="ExternalInput")
    bqk_d = nc.dram_tensor("bqk", [2 * HPC * DH], F32, kind="ExternalInput")
    bv_d = nc.dram_tensor("bv", [1, HPC * DH], F32, kind="ExternalInput")
    vones_d = nc.dram_tensor("vones", [P, LCH * HPC], F32R, kind="ExternalInput")
    out_d = nc.dram_tensor("out_part", [C, L], F32, kind="ExternalOutput")

    with tile.TileContext(nc) as tc:
        with (
            tc.tile_pool(name="persist", bufs=1) as pp,
            tc.tile_pool(name="small", bufs=4) as sp,
        ):
            # ---- persistent tiles ----
            wqk_sb = pp.tile([P, KC, 2 * HPC * DH], F32R)    # [128,4,512]
            wv_sb = pp.tile([P, KC, HPC * DH], F32R)         # [128,4,256]
            wp_sb = pp.tile([P, 2, C], F32R)                 # [128,2,512]
            bqk_sb = pp.tile([P, 4], F32)
            bvb_sb = pp.tile([P, HPC * DH], F32)             # broadcast v bias
            qkT_sb = pp.tile([P, 4, L], F32R)                # q^T,k^T [c_out,l]
            v_sb = pp.tile([P, LCH, HPC, DH + 1], F32R)      # v + ones col
            onT_sb = pp.tile([P, 2, L], F32R)                # normalized o^T
            ones_sb = pp.tile([P, 1], F32R)
            eps_sb = sp.tile([1, 1], F32)

            nc.sync.dma_start(wqk_sb[:], wqk_d.rearrange("(kc p) m -> p kc m", p=P))
            nc.sync.dma_start(wv_sb[:], wv_d.rearrange("(kc p) m -> p kc m", p=P))
            nc.sync.dma_start(wp_sb[:], wp_d.rearrange("(kc p) m -> p kc m", p=P))
            nc.sync.dma_start(bqk_sb[:], bqk_d.rearrange("(mc p) -> p mc", p=P))
            nc.sync.dma_start(bvb_sb[:], bv_d[0:1, :].partition_broadcast(P).opt())
            nc.sync.dma_start(ones_sb[:], vones_d[:, 0:1])
            nc.vector.memset(eps_sb[:], EPS)
            nc.sync.dma_start(
                v_sb[:, :, :, DH:DH + 1],
                vones_d.rearrange("p (lc h one) -> p lc h one", h=HPC, one=1),
            )

            # ================= phase A: LN stats + QKV GEMMs =================
            with (
                tc.tile_pool(name="xa", bufs=2) as xa,
                tc.tile_pool(name="stats", bufs=2) as st,
                tc.tile_pool(name="hnp", bufs=3) as hnp,
                tc.tile_pool(name="psumA", bufs=2, space="PSUM") as psA,
            ):
                for s in range(NSTRIP):
                    ls = bass.ts(s, STRIP)
                    x_sb = xa.tile([P, KC, STRIP], F32R, tag="x")
                    nc.sync.dma_start(
                        x_sb[:], x_d[:, ls].rearrange("(kc p) l -> p kc l", p=P)
                    )
                    xf = x_sb.bitcast(F32)
                    x2 = xa.tile([P, KC, STRIP], F32R, tag="x2")
                    nc.gpsimd.tensor_mul(x2[:], xf[:], xf[:])

                    ps_sum = psA.tile([1, STRIP], F32, tag="stat_sum")
                    ps_sq = psA.tile([1, STRIP], F32, tag="stat_sq")
                    for kc in range(KC):
                        nc.tensor.matmul(
                            ps_sum[:], ones_sb[:], x_sb[:, kc, :],
                            start=(kc == 0), stop=(kc == KC - 1),
                        )
                    for kc in range(KC):
                        nc.tensor.matmul(
                            ps_sq[:], ones_sb[:], x2[:, kc, :],
                            start=(kc == 0), stop=(kc == KC - 1),
                        )

                    mu = st.tile([1, STRIP], F32, tag="mu")
                    t2 = st.tile([1, STRIP], F32, tag="t2")
                    var = st.tile([1, STRIP], F32, tag="var")
                    rstd = st.tile([1, STRIP], F32, tag="rstd")
                    murstd = st.tile([1, STRIP], F32, tag="murstd")
                    nc.vector.tensor_scalar_mul(mu[:], ps_sum[:], 1.0 / C)
                    nc.vector.tensor_mul(t2[:], mu[:], mu[:])
                    nc.vector.scalar_tensor_tensor(
                        var[:], ps_sq[:], 1.0 / C, t2[:],
                        op0=ALU.mult, op1=ALU.subtract,
                    )
                    nc.scalar.activation(var[:], var[:], ACTF.Sqrt, bias=eps_sb[:])
                    nc.vector.reciprocal(rstd[:], var[:])
                    nc.vector.tensor_mul(murstd[:], mu[:], rstd[:])

                    rstd_b = st.tile([P, STRIP], F32, tag="rstd_b")
                    murstd_b = st.tile([P, STRIP], F32, tag="murstd_b")
                    nc.gpsimd.partition_broadcast(rstd_b[:], rstd[0:1, :])
                    nc.gpsimd.partition_broadcast(murstd_b[:], murstd[0:1, :])

                    # hn = x*rstd - mu*rstd  (LN with gamma/beta folded on host)
                    hn = hnp.tile([P, KC, STRIP], F32R, tag="hn")
                    hf = hn.bitcast(F32)
                    for kc in range(KC):
                        nc.gpsimd.tensor_mul(hn[:, kc, :], xf[:, kc, :], rstd_b[:])
                    for kc in range(KC):
                        nc.vector.tensor_sub(
                            hn[:, kc, :], hf[:, kc, :], murstd_b[:]
                        )

                    # q^T,k^T GEMM: out [c_out, l]
                    for mc in range(4):
                        pqk = psA.tile([P, STRIP], F32, tag="qk")
                        for kc in range(KC):
                            nc.tensor.matmul(
                                pqk[:], wqk_sb[:, kc, bass.ts(mc, P)], hn[:, kc, :],
                                start=(kc == 0), stop=(kc == KC - 1),
                            )
                        nc.scalar.activation(
                            qkT_sb[:, mc, ls], pqk[:], ACTF.Identity,
                            bias=bqk_sb[:, mc:mc + 1],
                        )

                    # v GEMM: out [l, d]
                    for lc in range(STRIP // P):
                        lg = s * (STRIP // P) + lc
                        pv = psA.tile([P, HPC * DH], F32, tag="v")
                        for kc in range(KC):
                            nc.tensor.matmul(
                                pv[:], hn[:, kc, bass.ts(lc, P)], wv_sb[:, kc, :],
                                start=(kc == 0), stop=(kc == KC - 1),
                            )
                        nc.vector.tensor_add(
                            v_sb[:, lg, :, 0:DH],
                            pv.rearrange("p (h d) -> p h d", h=HPC),
                            bvb_sb.rearrange("p (h d) -> p h d", h=HPC),
                        )

            # ================= phase B: attention per head =================
            with (
                tc.tile_pool(name="expp", bufs=3) as ep,
                tc.tile_pool(name="psumB", bufs=2, space="PSUM") as psB,
                tc.tile_pool(name="psumO", bufs=4, space="PSUM") as psO,
            ):
                for h in range(HPC):
                    po = (h % 2) * DH
                    qT = qkT_sb[po:po + DH, h // 2, :]
                    kT = qkT_sb[po:po + DH, 2 + h // 2, :]
                    oT = [psO.tile([DH + 1, STRIP], F32, tag="oT", name=f"oT{h}_{i}")
                          for i in range(4)]
                    for lk in range(LCH):
                        ex = ep.tile([P, L], F32R, tag="expT")
                        for half in range(2):
                            pst = psB.tile([P, 1024], F32, tag="sT")
                            for q2 in range(2):
                                nc.tensor.matmul(
                                    pst[:, bass.ts(q2, 512)],
                                    kT[:, bass.ts(lk, P)],
                                    qT[:, bass.ds(half * 1024 + q2 * 512, 512)],
                                    start=True, stop=True,
                                )
                            nc.scalar.activation(
                                ex[:, bass.ts(half, 1024)], pst[:],
                                ACTF.Exp, scale=SCALE,
                            )
                        for s in range(4):
                            nc.tensor.matmul(
                                oT[s][:], v_sb[:, lk, h, :], ex[:, bass.ts(s, STRIP)],
                                start=(lk == 0), stop=(lk == LCH - 1),
                            )
                    # normalize: onT[d, l] = oT[d, l] / Z[l]
                    for s in range(4):
                        rz = sp.tile([1, STRIP], F32, tag="rz")
                        rz_b = sp.tile([DH, STRIP], F32, tag="rz_b")
                        nc.vector.reciprocal(rz[:], oT[s][DH:DH + 1, :])
                        nc.gpsimd.partition_broadcast(rz_b[:], rz[0:1, :])
                        nc.vector.tensor_mul(
                            onT_sb[po:po + DH, h // 2, bass.ts(s, STRIP)],
                            oT[s][0:DH, :], rz_b[:],
                        )

            # ================= phase C: proj partial =================
            with (
                tc.tile_pool(name="outp", bufs=2) as op_,
                tc.tile_pool(name="psumC", bufs=2, space="PSUM") as psC,
            ):
                for s in range(NSTRIP):
                    ls = bass.ts(s, STRIP)
                    ot = op_.tile([P, 4, STRIP], F32, tag="out")
                    for mc in range(4):
                        ppj = psC.tile([P, STRIP], F32, tag="proj")
                        for kc in range(2):
                            nc.tensor.matmul(
                                ppj[:], wp_sb[:, kc, bass.ts(mc, P)],
                                onT_sb[:, kc, ls],
                                start=(kc == 0), stop=(kc == 1),
                            )
                        nc.vector.tensor_copy(ot[:, mc, :], ppj[:])
                    nc.sync.dma_start(
                        out_d[:, ls].rearrange("(mc p) l -> p mc l", p=P), ot[:]
                    )

    nc.compile()
    return nc


_NC = None


def _get_nc():
    global _NC
    if _NC is None:
        _NC = build_nc()
    return _NC


def make_core_inputs(x, ln_gamma, ln_beta, w_qkv, b_qkv, w_proj, b_proj):
    """Host-side shard prep. Folds ln_gamma/ln_beta into the QKV weights."""
    x = np.asarray(x, np.float32)
    g_ = np.asarray(ln_gamma, np.float32)
    be = np.asarray(ln_beta, np.float32)
    w_qkv = np.asarray(w_qkv, np.float32)
    b_qkv = np.asarray(b_qkv, np.float32)
    w_proj = np.asarray(w_proj, np.float32)
    in_maps = []
    for core in range(8):
        b = core // 2
        gr = core % 2
        rs = slice(gr * HPC * DH, (gr + 1) * HPC * DH)
        wq, wk, wv = (w_qkv[i * C:(i + 1) * C][rs] for i in range(3))
        bq, bk, bv = (b_qkv[i * C:(i + 1) * C][rs] for i in range(3))
        # gamma folds into W columns; beta folds into the bias
        wqg, wkg, wvg = (w * g_[None, :] for w in (wq, wk, wv))
        bq = bq + wq @ be
        bk = bk + wk @ be
        bv = bv + wv @ be
        in_maps.append({
            "x_loc": np.ascontiguousarray(x[b]),
            "wqkT": np.ascontiguousarray(np.concatenate([wqg, wkg], 0).T),
            "wvT": np.ascontiguousarray(wvg.T),
            "wprojT": np.ascontiguousarray(w_proj[:, rs].T),
            "bqk": np.ascontiguousarray(np.concatenate([bq, bk])),
            "bv": np.ascontiguousarray(bv[None, :]),
            "vones": np.ones((P, LCH * HPC), np.float32),
        })
    return in_maps


def combine(partials, x, b_proj):
    out = np.empty((B, C, L), np.float32)
    for b in range(B):
        out[b] = (partials[2 * b] + partials[2 * b + 1]
                  + np.asarray(b_proj, np.float32)[:, None]
                  + np.asarray(x, np.float32)[b])
    return out


def run_cores(in_maps, trace=False, **kw):
    nc = _get_nc()
    return run_bass_kernel_spmd(nc, in_maps, core_ids=list(range(8)),
                                trace=trace, **kw)


def kernel(**inputs):
    in_maps = make_core_inputs(**inputs)
    res = run_cores(in_maps)
    partials = [r["out_part"] for r in res.results]
    return combine(partials, inputs["x"], inputs["b_proj"])
